# revision 11
# baseline (speedup 1.0000x reference)
"""4-bit groupwise-quantized linear layer (CLinear) on 8 Trainium2 NeuronCores.

Full-input contract: kernel(**inputs) takes the unsharded numpy inputs
  x      [4, 2048, 4096] fp32
  packed [4096, 64, 32]  int32 (byte values; hi nibble = first half of each
                                quant group, lo nibble = second half)
  mn     [4096, 64, 1]   fp32
  scale  [4096, 64, 1]   fp32
  bias   [4096]          fp32
and returns out[4, 2048, 4096] fp32 = x @ dequant(packed, mn, scale).T + bias.

Sharding: 2D grid over 8 cores — 2 token-row groups x 4 out-column groups.
Core (r, c) computes out[r*4096:(r+1)*4096, c*1024:(c+1)*1024] (transposed on
device, transposed back during host assembly). No collectives.

v4 design (per core: M=4096 tokens, K=4096, N=1024 out features):
  - x is transposed to [K, M] and cast to bf16 on the host (layout staging),
    so the device streams matmul-ready [k, m] tiles with zero on-device
    transposes or dtype-conversion passes.
  - packed is host-repacked int32 -> uint8 (lossless byte view). Nibble
    extraction + affine dequant (v * 1/scale + mn) run on DVE, n-tile
    granular, bf16 out, DMA-transposed into resident [k, n] weight tiles.
  - matmul: weights stationary, 512-token moving blocks, fp32 PSUM
    accumulation over 32 k-tiles; bias folded into the scalar-engine PSUM
    eviction.
  - Engine queues partitioned so nothing blocks head-of-line: sync=SP
    streams packed[ntile 0] + x blocks (triple-buffered), GPSIMD loads the
    small constants + remaining packed tiles, scalar owns weight-transposes
    + PSUM evictions + output stores, DVE owns dequant math.
  - Phase 1 walks n-tiles feeding each fresh weight tile two token blocks
    (PE demand ~13.6us/ntile ~= dequant supply ~11.5us/ntile); phase 2
    streams the remaining token blocks against resident weights.
  - A burst of dummy 128-row matmuls at t~=7us lifts the PE HAM clock gate
    to 2.4 GHz before the real matmul stream begins.
"""

import sys
from contextlib import ExitStack

import numpy as np

if "/opt/trn_rl_repo" not in sys.path:
    sys.path.insert(0, "/opt/trn_rl_repo")

import concourse.mybir as mybir
import concourse.tile as tile
from concourse import bacc
from concourse.bass_utils import run_bass_kernel_spmd

FP32 = mybir.dt.float32
BF16 = mybir.dt.bfloat16
U8 = mybir.dt.uint8
P = 128
GS = 64  # quant group size

# problem shape (hardcoded)
B, S, IN, OUT = 4, 2048, 4096, 4096
R_SHARDS, C_SHARDS = 2, 4
M_CORE = B * S // R_SHARDS      # 4096 tokens per core
N_CORE = OUT // C_SHARDS        # 1024 out features per core
MB = 512                        # tokens per matmul block


def _emit_kernel(tc, outs, ins, M, K, N, MB=512):
    nc = tc.nc
    ctx = ExitStack()
    G = K // GS               # 64 quant groups along k
    KT = K // P               # 32 k-tiles
    NT = N // P               # 8 n-tiles
    QT = M // MB              # 8 token blocks
    assert K % P == 0 and N % P == 0 and M % MB == 0

    x_d = ins["x"]            # [K, M] bf16 (host-transposed)
    pk_d = ins["packed"]      # [N, G, 32] u8
    mn_d = ins["mn"]          # [N, G] fp32
    sc_d = ins["scale"]       # [N, G] fp32
    b_d = ins["bias"]         # [1, N] fp32
    out_d = outs["out"]       # [N, M] fp32  (transposed)

    with ctx:
        const = ctx.enter_context(tc.tile_pool(name="const", bufs=1))
        wres = ctx.enter_context(tc.tile_pool(name="wres", bufs=NT))
        deq = ctx.enter_context(tc.tile_pool(name="deq", bufs=2))
        xin = ctx.enter_context(tc.tile_pool(name="xin", bufs=3))
        outp = ctx.enter_context(tc.tile_pool(name="outp", bufs=4))
        psum = ctx.enter_context(tc.tile_pool(name="psum", bufs=4, space="PSUM"))
        warm = ctx.enter_context(tc.tile_pool(name="warm", bufs=1))

        # --- PE clock warmup: ~5us of dummy matmuls with no data deps so the
        # HAM activity window sees a busy PE and lifts the clock gate to
        # 2.4 GHz before the real stream starts.
        wsrc = warm.tile([P, P], BF16)
        nc.vector.memset(wsrc[:], 0)
        for _ in range(64):
            nc.tensor.ldweights(wsrc[:])

        def load_pk(nt, eng):
            pk_t = deq.tile([P, G, 32], U8, tag="pk", bufs=1)
            eng.dma_start(out=pk_t[:], in_=pk_d[nt * P:(nt + 1) * P])
            return pk_t

        # packed for n-tile 0 heads the fast SP queue (it gates the whole
        # weight pipeline); the rest trickle in on GPSIMD. The small
        # constants follow on SP (their semaphores gate the first dequant);
        # bias via GPSIMD SWDGE (needed late).
        pk0 = load_pk(0, nc.sync)
        sc_all = const.tile([P, NT, G], FP32)
        nc.sync.dma_start(out=sc_all[:],
                          in_=sc_d[:].rearrange("(t p) g -> p t g", p=P))
        mn_all = const.tile([P, NT, G], FP32)
        nc.sync.dma_start(out=mn_all[:],
                          in_=mn_d[:].rearrange("(t p) g -> p t g", p=P))
        # bias laid out [P, NT]: column nt holds bias[nt*128:(nt+1)*128]
        bias_pt = const.tile([P, NT], FP32)
        nc.gpsimd.dma_start(out=bias_pt[:],
                            in_=b_d[:].rearrange("1 (t p) -> p t", p=P))

        def load_x(q):
            xq = xin.tile([P, KT, MB], BF16, tag="xq")
            nc.sync.dma_start(
                out=xq[:],
                in_=x_d[:, q * MB:(q + 1) * MB].rearrange(
                    "(kt p) m -> p kt m", p=P))
            return xq

        xqs = [load_x(0), load_x(1), load_x(2)]

        # absorb the one-time first-DMA-transpose init penalty (~8us) off
        # the weight-pipeline critical path; emitted after the DMA loads so
        # it does not head their shared in-order DMA queue
        wdst = warm.tile([P, P], BF16)
        nc.scalar.dma_start_transpose(wdst[:], wsrc[:])

        def dequant_ntile(nt):
            pk_t = pk0 if nt == 0 else load_pk(nt, nc.gpsimd)
            vals = deq.tile([P, G, GS], U8, tag="vals")
            nc.vector.tensor_scalar(
                vals[:, :, 0:32], pk_t[:], 4, None,
                mybir.AluOpType.logical_shift_right)
            nc.vector.tensor_scalar(
                vals[:, :, 32:64], pk_t[:], 15, None,
                mybir.AluOpType.bitwise_and)
            inv_t = deq.tile([P, G], BF16, tag="inv")
            with nc.allow_low_precision(reason="4-bit dequant; bf16 ample"):
                nc.vector.reciprocal(inv_t[:], sc_all[:, nt])
            wbf = deq.tile([P, G, GS], BF16, tag="wbf")
            inv_b = inv_t[:].unsqueeze(2).broadcast_to([P, G, GS])
            nc.vector.tensor_tensor(wbf[:], vals[:], inv_b,
                                    mybir.AluOpType.mult)
            mn_b = mn_all[:, nt].unsqueeze(2).broadcast_to([P, G, GS])
            nc.vector.tensor_tensor(wbf[:], wbf[:], mn_b,
                                    mybir.AluOpType.add)
            wt = wres.tile([P, KT, P], BF16, tag="wt")
            nc.scalar.dma_start_transpose(
                wt[:], wbf[:].rearrange("p g j -> p (g j)"))
            return wt

        def mm_block(wt, xq, q, nt):
            pt = psum.tile([P, MB], FP32, tag="pt")
            for kt in range(KT):
                nc.tensor.matmul(pt[:], lhsT=wt[:, kt, :],
                                 rhs=xq[:, kt, :],
                                 start=(kt == 0), stop=(kt == KT - 1))
            ot = outp.tile([P, MB], FP32, tag="ot")
            nc.scalar.activation(ot[:], pt[:],
                                 mybir.ActivationFunctionType.Identity,
                                 bias=bias_pt[:, nt:nt + 1])
            nc.scalar.dma_start(
                out=out_d[nt * P:(nt + 1) * P, q * MB:(q + 1) * MB],
                in_=ot[:])

        # Phase 1: n-tile-major over the first two token blocks (PE consumes
        # ~13.6us per fresh weight tile, matching the dequant cadence).
        wts = []
        for nt in range(NT):
            wts.append(dequant_ntile(nt))
            mm_block(wts[nt], xqs[0], 0, nt)
            mm_block(wts[nt], xqs[1], 1, nt)

        # Phase 2: token-block-major over the resident weight tiles.
        xq_cur = xqs[2]
        for q in range(2, QT):
            xq_next = load_x(q + 1) if q + 1 < QT else None
            for nt in range(NT):
                mm_block(wts[nt], xq_cur, q, nt)
            xq_cur = xq_next


_CACHED = {}


def _build():
    if "nc" in _CACHED:
        return _CACHED["nc"]
    nc = bacc.Bacc("TRN2", target_bir_lowering=False, debug=False)
    tensors = {
        "x": nc.dram_tensor("x", [IN, M_CORE], BF16, kind="ExternalInput"),
        "packed": nc.dram_tensor("packed", [N_CORE, IN // GS, GS // 2], U8,
                                 kind="ExternalInput"),
        "mn": nc.dram_tensor("mn", [N_CORE, IN // GS], FP32,
                             kind="ExternalInput"),
        "scale": nc.dram_tensor("scale", [N_CORE, IN // GS], FP32,
                                kind="ExternalInput"),
        "bias": nc.dram_tensor("bias", [1, N_CORE], FP32,
                               kind="ExternalInput"),
        "out": nc.dram_tensor("out", [N_CORE, M_CORE], FP32,
                              kind="ExternalOutput"),
    }
    ins = {k: tensors[k].ap() for k in ("x", "packed", "mn", "scale", "bias")}
    outs = {"out": tensors["out"].ap()}
    with tile.TileContext(nc) as tc:
        _emit_kernel(tc, outs, ins, M=M_CORE, K=IN, N=N_CORE, MB=MB)
    nc.compile()
    _CACHED["nc"] = nc
    return nc


def kernel(x, packed, mn, scale, bias, _trace=False, _trace_kwargs=None):
    import ml_dtypes

    nc = _build()

    xf = np.asarray(x, np.float32).reshape(B * S, IN)
    xT = [
        np.ascontiguousarray(
            xf[r * M_CORE:(r + 1) * M_CORE].T).astype(ml_dtypes.bfloat16)
        for r in range(R_SHARDS)
    ]
    pk_u8 = np.asarray(packed).astype(np.uint8)  # byte values 0..255, lossless
    in_maps = []
    for r in range(R_SHARDS):
        for c in range(C_SHARDS):
            ns = slice(c * N_CORE, (c + 1) * N_CORE)
            in_maps.append({
                "x": xT[r],
                "packed": np.ascontiguousarray(pk_u8[ns]),
                "mn": np.ascontiguousarray(mn[ns, :, 0]),
                "scale": np.ascontiguousarray(scale[ns, :, 0]),
                "bias": np.ascontiguousarray(
                    bias[ns].reshape(1, N_CORE).astype(np.float32)),
            })

    res = run_bass_kernel_spmd(
        nc, in_maps, core_ids=list(range(R_SHARDS * C_SHARDS)),
        trace=_trace, **(_trace_kwargs or {}))

    out = np.empty((B * S, OUT), np.float32)
    for r in range(R_SHARDS):
        for c in range(C_SHARDS):
            shard = res.results[r * C_SHARDS + c]["out"]  # [N_CORE, M_CORE]
            out[r * M_CORE:(r + 1) * M_CORE,
                c * N_CORE:(c + 1) * N_CORE] = shard.T
    kernel.last_exec_time_ns = res.exec_time_ns
    kernel.last_profile = res.profile_json
    return out.reshape(B, S, OUT)


# revision 14
# speedup vs baseline: 1.0470x; 1.0470x over previous
"""4-bit groupwise-quantized linear layer (CLinear) on 8 Trainium2 NeuronCores.

Full-input contract: kernel(**inputs) takes the unsharded numpy inputs
  x      [4, 2048, 4096] fp32
  packed [4096, 64, 32]  int32 (byte values; hi nibble = first half of each
                                quant group, lo nibble = second half)
  mn     [4096, 64, 1]   fp32
  scale  [4096, 64, 1]   fp32
  bias   [4096]          fp32
and returns out[4, 2048, 4096] fp32 = x @ dequant(packed, mn, scale).T + bias.

Sharding: 2D grid over 8 cores — 2 token-row groups x 4 out-column groups.
Core (r, c) computes out[r*4096:(r+1)*4096, c*1024:(c+1)*1024] (transposed on
device, transposed back during host assembly). No collectives.

v4 design (per core: M=4096 tokens, K=4096, N=1024 out features):
  - x is transposed to [K, M] and cast to bf16 on the host (layout staging),
    so the device streams matmul-ready [k, m] tiles with zero on-device
    transposes or dtype-conversion passes.
  - packed is host-repacked int32 -> uint8 (lossless byte view). Nibble
    extraction + affine dequant (v * 1/scale + mn) run on DVE, n-tile
    granular, bf16 out, DMA-transposed into resident [k, n] weight tiles.
  - matmul: weights stationary, 512-token moving blocks, fp32 PSUM
    accumulation over 32 k-tiles; bias folded into the scalar-engine PSUM
    eviction.
  - Engine queues partitioned so nothing blocks head-of-line: sync=SP
    streams packed[ntile 0] + x blocks (triple-buffered), GPSIMD loads the
    small constants + remaining packed tiles, scalar owns weight-transposes
    + PSUM evictions + output stores, DVE owns dequant math.
  - Phase 1 walks n-tiles feeding each fresh weight tile two token blocks
    (PE demand ~13.6us/ntile ~= dequant supply ~11.5us/ntile); phase 2
    streams the remaining token blocks against resident weights.
  - A burst of dummy 128-row matmuls at t~=7us lifts the PE HAM clock gate
    to 2.4 GHz before the real matmul stream begins.
"""

import sys
from contextlib import ExitStack

import numpy as np

if "/opt/trn_rl_repo" not in sys.path:
    sys.path.insert(0, "/opt/trn_rl_repo")

import concourse.mybir as mybir
import concourse.tile as tile
from concourse import bacc
from concourse.bass_utils import run_bass_kernel_spmd

FP32 = mybir.dt.float32
BF16 = mybir.dt.bfloat16
U8 = mybir.dt.uint8
P = 128
GS = 64  # quant group size

# problem shape (hardcoded)
B, S, IN, OUT = 4, 2048, 4096, 4096
R_SHARDS, C_SHARDS = 2, 4
M_CORE = B * S // R_SHARDS      # 4096 tokens per core
N_CORE = OUT // C_SHARDS        # 1024 out features per core
MB = 512                        # tokens per matmul block


def _emit_kernel(tc, outs, ins, M, K, N, MB=512):
    nc = tc.nc
    ctx = ExitStack()
    G = K // GS               # 64 quant groups along k
    KT = K // P               # 32 k-tiles
    NT = N // P               # 8 n-tiles
    QT = M // MB              # 8 token blocks
    assert K % P == 0 and N % P == 0 and M % MB == 0

    x_d = ins["x"]            # [K, M] bf16 (host-transposed)
    pk_d = ins["packed"]      # [N, G, 32] u8
    mn_d = ins["mn"]          # [N, G] fp32
    sc_d = ins["scale"]       # [N, G] fp32
    b_d = ins["bias"]         # [1, N] fp32
    out_d = outs["out"]       # [N, M] fp32  (transposed)

    with ctx:
        const = ctx.enter_context(tc.tile_pool(name="const", bufs=1))
        wres = ctx.enter_context(tc.tile_pool(name="wres", bufs=NT))
        deq = ctx.enter_context(tc.tile_pool(name="deq", bufs=2))
        xin = ctx.enter_context(tc.tile_pool(name="xin", bufs=3))
        outp = ctx.enter_context(tc.tile_pool(name="outp", bufs=4))
        psum = ctx.enter_context(tc.tile_pool(name="psum", bufs=4, space="PSUM"))
        warm = ctx.enter_context(tc.tile_pool(name="warm", bufs=1))

        # --- PE clock warmup: ~5us of dummy matmuls with no data deps so the
        # HAM activity window sees a busy PE and lifts the clock gate to
        # 2.4 GHz before the real stream starts.
        wsrc = warm.tile([P, P], BF16)
        nc.vector.memset(wsrc[:], 0)
        for _ in range(64):
            nc.tensor.ldweights(wsrc[:])

        def load_pk(nt, eng):
            pk_t = deq.tile([P, G, 32], U8, tag="pk")
            eng.dma_start(out=pk_t[:], in_=pk_d[nt * P:(nt + 1) * P])
            return pk_t

        # packed for n-tile 0 heads the fast SP queue (it gates the whole
        # weight pipeline); the rest trickle in on GPSIMD. The small
        # constants follow on SP (their semaphores gate the first dequant);
        # bias via GPSIMD SWDGE (needed late).
        pk0 = load_pk(0, nc.sync)
        sc_all = const.tile([P, NT, G], FP32)
        nc.sync.dma_start(out=sc_all[:],
                          in_=sc_d[:].rearrange("(t p) g -> p t g", p=P))
        mn_all = const.tile([P, NT, G], FP32)
        nc.sync.dma_start(out=mn_all[:],
                          in_=mn_d[:].rearrange("(t p) g -> p t g", p=P))
        # bias laid out [P, NT]: column nt holds bias[nt*128:(nt+1)*128]
        bias_pt = const.tile([P, NT], FP32)
        nc.gpsimd.dma_start(out=bias_pt[:],
                            in_=b_d[:].rearrange("1 (t p) -> p t", p=P))

        def load_x(q):
            xq = xin.tile([P, KT, MB], BF16, tag="xq")
            nc.sync.dma_start(
                out=xq[:],
                in_=x_d[:, q * MB:(q + 1) * MB].rearrange(
                    "(kt p) m -> p kt m", p=P))
            return xq

        # xq2 is loaded mid-phase-1: issuing it up front makes its 12us DMA
        # overlap (and slow) the first weight transpose on the critical path.
        xqs = [load_x(0), load_x(1), None]

        def dequant_ntile(nt):
            pk_t = pk0 if nt == 0 else load_pk(nt, nc.gpsimd)
            vals = deq.tile([P, G, GS], U8, tag="vals")
            nc.vector.tensor_scalar(
                vals[:, :, 0:32], pk_t[:], 4, None,
                mybir.AluOpType.logical_shift_right)
            nc.vector.tensor_scalar(
                vals[:, :, 32:64], pk_t[:], 15, None,
                mybir.AluOpType.bitwise_and)
            inv_t = deq.tile([P, G], BF16, tag="inv")
            with nc.allow_low_precision(reason="4-bit dequant; bf16 ample"):
                nc.vector.reciprocal(inv_t[:], sc_all[:, nt])
            wbf = deq.tile([P, G, GS], BF16, tag="wbf")
            inv_b = inv_t[:].unsqueeze(2).broadcast_to([P, G, GS])
            nc.vector.tensor_tensor(wbf[:], vals[:], inv_b,
                                    mybir.AluOpType.mult)
            mn_b = mn_all[:, nt].unsqueeze(2).broadcast_to([P, G, GS])
            nc.vector.tensor_tensor(wbf[:], wbf[:], mn_b,
                                    mybir.AluOpType.add)
            wt = wres.tile([P, KT, P], BF16, tag="wt")
            nc.scalar.dma_start_transpose(
                wt[:], wbf[:].rearrange("p g j -> p (g j)"))
            return wt

        def mm_block(wt, xq, q, nt):
            pt = psum.tile([P, MB], FP32, tag="pt")
            for kt in range(KT):
                nc.tensor.matmul(pt[:], lhsT=wt[:, kt, :],
                                 rhs=xq[:, kt, :],
                                 start=(kt == 0), stop=(kt == KT - 1))
            ot = outp.tile([P, MB], FP32, tag="ot")
            nc.scalar.activation(ot[:], pt[:],
                                 mybir.ActivationFunctionType.Identity,
                                 bias=bias_pt[:, nt:nt + 1])
            nc.scalar.dma_start(
                out=out_d[nt * P:(nt + 1) * P, q * MB:(q + 1) * MB],
                in_=ot[:])

        # Phase 1: n-tile-major over the first two token blocks (PE consumes
        # ~13.6us per fresh weight tile, matching the dequant cadence).
        wts = []
        for nt in range(NT):
            wts.append(dequant_ntile(nt))
            mm_block(wts[nt], xqs[0], 0, nt)
            mm_block(wts[nt], xqs[1], 1, nt)
            if nt == 3:
                xqs[2] = load_x(2)

        # Phase 2: token-block-major over the resident weight tiles.
        xq_cur = xqs[2]
        for q in range(2, QT):
            xq_next = load_x(q + 1) if q + 1 < QT else None
            for nt in range(NT):
                mm_block(wts[nt], xq_cur, q, nt)
            xq_cur = xq_next


_CACHED = {}


def _build():
    if "nc" in _CACHED:
        return _CACHED["nc"]
    nc = bacc.Bacc("TRN2", target_bir_lowering=False, debug=False)
    tensors = {
        "x": nc.dram_tensor("x", [IN, M_CORE], BF16, kind="ExternalInput"),
        "packed": nc.dram_tensor("packed", [N_CORE, IN // GS, GS // 2], U8,
                                 kind="ExternalInput"),
        "mn": nc.dram_tensor("mn", [N_CORE, IN // GS], FP32,
                             kind="ExternalInput"),
        "scale": nc.dram_tensor("scale", [N_CORE, IN // GS], FP32,
                                kind="ExternalInput"),
        "bias": nc.dram_tensor("bias", [1, N_CORE], FP32,
                               kind="ExternalInput"),
        "out": nc.dram_tensor("out", [N_CORE, M_CORE], FP32,
                              kind="ExternalOutput"),
    }
    ins = {k: tensors[k].ap() for k in ("x", "packed", "mn", "scale", "bias")}
    outs = {"out": tensors["out"].ap()}
    with tile.TileContext(nc) as tc:
        _emit_kernel(tc, outs, ins, M=M_CORE, K=IN, N=N_CORE, MB=MB)
    nc.compile()
    _CACHED["nc"] = nc
    return nc


def kernel(x, packed, mn, scale, bias, _trace=False, _trace_kwargs=None):
    import ml_dtypes

    nc = _build()

    xf = np.asarray(x, np.float32).reshape(B * S, IN)
    xT = [
        np.ascontiguousarray(
            xf[r * M_CORE:(r + 1) * M_CORE].T).astype(ml_dtypes.bfloat16)
        for r in range(R_SHARDS)
    ]
    pk_u8 = np.asarray(packed).astype(np.uint8)  # byte values 0..255, lossless
    in_maps = []
    for r in range(R_SHARDS):
        for c in range(C_SHARDS):
            ns = slice(c * N_CORE, (c + 1) * N_CORE)
            in_maps.append({
                "x": xT[r],
                "packed": np.ascontiguousarray(pk_u8[ns]),
                "mn": np.ascontiguousarray(mn[ns, :, 0]),
                "scale": np.ascontiguousarray(scale[ns, :, 0]),
                "bias": np.ascontiguousarray(
                    bias[ns].reshape(1, N_CORE).astype(np.float32)),
            })

    res = run_bass_kernel_spmd(
        nc, in_maps, core_ids=list(range(R_SHARDS * C_SHARDS)),
        trace=_trace, **(_trace_kwargs or {}))

    out = np.empty((B * S, OUT), np.float32)
    for r in range(R_SHARDS):
        for c in range(C_SHARDS):
            shard = res.results[r * C_SHARDS + c]["out"]  # [N_CORE, M_CORE]
            out[r * M_CORE:(r + 1) * M_CORE,
                c * N_CORE:(c + 1) * N_CORE] = shard.T
    kernel.last_exec_time_ns = res.exec_time_ns
    kernel.last_profile = res.profile_json
    return out.reshape(B, S, OUT)


# revision 16
# speedup vs baseline: 1.0478x; 1.0008x over previous
"""4-bit groupwise-quantized linear layer (CLinear) on 8 Trainium2 NeuronCores.

Full-input contract: kernel(**inputs) takes the unsharded numpy inputs
  x      [4, 2048, 4096] fp32
  packed [4096, 64, 32]  int32 (byte values; hi nibble = first half of each
                                quant group, lo nibble = second half)
  mn     [4096, 64, 1]   fp32
  scale  [4096, 64, 1]   fp32
  bias   [4096]          fp32
and returns out[4, 2048, 4096] fp32 = x @ dequant(packed, mn, scale).T + bias.

Sharding: 2D grid over 8 cores — 2 token-row groups x 4 out-column groups.
Core (r, c) computes out[r*4096:(r+1)*4096, c*1024:(c+1)*1024] (transposed on
device, transposed back during host assembly). No collectives.

v4 design (per core: M=4096 tokens, K=4096, N=1024 out features):
  - x is transposed to [K, M] and cast to bf16 on the host (layout staging),
    so the device streams matmul-ready [k, m] tiles with zero on-device
    transposes or dtype-conversion passes.
  - packed is host-repacked int32 -> uint8 (lossless byte view). Nibble
    extraction + affine dequant (v * 1/scale + mn) run on DVE, n-tile
    granular, bf16 out, DMA-transposed into resident [k, n] weight tiles.
  - matmul: weights stationary, 512-token moving blocks, fp32 PSUM
    accumulation over 32 k-tiles; bias folded into the scalar-engine PSUM
    eviction.
  - Engine queues partitioned so nothing blocks head-of-line: sync=SP
    streams packed[ntile 0] + x blocks (triple-buffered), GPSIMD loads the
    small constants + remaining packed tiles, scalar owns weight-transposes
    + PSUM evictions + output stores, DVE owns dequant math.
  - Phase 1 walks n-tiles feeding each fresh weight tile two token blocks
    (PE demand ~13.6us/ntile ~= dequant supply ~11.5us/ntile); phase 2
    streams the remaining token blocks against resident weights.
  - A burst of dummy 128-row matmuls at t~=7us lifts the PE HAM clock gate
    to 2.4 GHz before the real matmul stream begins.
"""

import sys
from contextlib import ExitStack

import numpy as np

if "/opt/trn_rl_repo" not in sys.path:
    sys.path.insert(0, "/opt/trn_rl_repo")

import concourse.mybir as mybir
import concourse.tile as tile
from concourse import bacc
from concourse.bass_utils import run_bass_kernel_spmd

FP32 = mybir.dt.float32
BF16 = mybir.dt.bfloat16
U8 = mybir.dt.uint8
P = 128
GS = 64  # quant group size

# problem shape (hardcoded)
B, S, IN, OUT = 4, 2048, 4096, 4096
R_SHARDS, C_SHARDS = 2, 4
M_CORE = B * S // R_SHARDS      # 4096 tokens per core
N_CORE = OUT // C_SHARDS        # 1024 out features per core
MB = 512                        # tokens per matmul block


def _emit_kernel(tc, outs, ins, M, K, N, MB=512):
    nc = tc.nc
    ctx = ExitStack()
    G = K // GS               # 64 quant groups along k
    KT = K // P               # 32 k-tiles
    NT = N // P               # 8 n-tiles
    QT = M // MB              # 8 token blocks
    assert K % P == 0 and N % P == 0 and M % MB == 0

    x_d = ins["x"]            # [K, M] bf16 (host-transposed)
    pk_d = ins["packed"]      # [N, G, 32] u8
    mn_d = ins["mn"]          # [N, G] fp32
    sc_d = ins["scale"]       # [N, G] fp32
    b_d = ins["bias"]         # [1, N] fp32
    out_d = outs["out"]       # [N, M] fp32  (transposed)

    with ctx:
        const = ctx.enter_context(tc.tile_pool(name="const", bufs=1))
        wres = ctx.enter_context(tc.tile_pool(name="wres", bufs=NT))
        deq = ctx.enter_context(tc.tile_pool(name="deq", bufs=2))
        xin = ctx.enter_context(tc.tile_pool(name="xin", bufs=3))
        outp = ctx.enter_context(tc.tile_pool(name="outp", bufs=4))
        psum = ctx.enter_context(tc.tile_pool(name="psum", bufs=4, space="PSUM"))
        warm = ctx.enter_context(tc.tile_pool(name="warm", bufs=1))

        # --- PE clock warmup: ~5us of dummy matmuls with no data deps so the
        # HAM activity window sees a busy PE and lifts the clock gate to
        # 2.4 GHz before the real stream starts.
        wsrc = warm.tile([P, P], BF16)
        nc.vector.memset(wsrc[:], 0)
        for _ in range(64):
            nc.tensor.ldweights(wsrc[:])
        # preload the activation table during the preamble so the one-time
        # ACT_TABLE_LOAD (~1.3us) is not scheduled ahead of the first weight
        # transpose on the critical path
        wact = warm.tile([P, P], BF16)
        nc.scalar.activation(wact[:], wsrc[:],
                             mybir.ActivationFunctionType.Identity)

        def load_pk(nt, eng):
            pk_t = deq.tile([P, G, 32], U8, tag="pk")
            eng.dma_start(out=pk_t[:], in_=pk_d[nt * P:(nt + 1) * P])
            return pk_t

        # packed for n-tile 0 heads the fast SP queue (it gates the whole
        # weight pipeline); the rest trickle in on GPSIMD. The small
        # constants follow on SP (their semaphores gate the first dequant);
        # bias via GPSIMD SWDGE (needed late).
        pk0 = load_pk(0, nc.sync)
        sc_all = const.tile([P, NT, G], FP32)
        nc.sync.dma_start(out=sc_all[:],
                          in_=sc_d[:].rearrange("(t p) g -> p t g", p=P))
        mn_all = const.tile([P, NT, G], FP32)
        nc.sync.dma_start(out=mn_all[:],
                          in_=mn_d[:].rearrange("(t p) g -> p t g", p=P))
        # bias laid out [P, NT]: column nt holds bias[nt*128:(nt+1)*128]
        bias_pt = const.tile([P, NT], FP32)
        nc.gpsimd.dma_start(out=bias_pt[:],
                            in_=b_d[:].rearrange("1 (t p) -> p t", p=P))

        def load_x(q):
            xq = xin.tile([P, KT, MB], BF16, tag="xq")
            nc.sync.dma_start(
                out=xq[:],
                in_=x_d[:, q * MB:(q + 1) * MB].rearrange(
                    "(kt p) m -> p kt m", p=P))
            return xq

        # xq2 is loaded mid-phase-1: issuing it up front makes its 12us DMA
        # overlap (and slow) the first weight transpose on the critical path.
        xqs = [load_x(0), load_x(1), None]

        def dequant_ntile(nt):
            pk_t = pk0 if nt == 0 else load_pk(nt, nc.gpsimd)
            vals = deq.tile([P, G, GS], U8, tag="vals")
            nc.vector.tensor_scalar(
                vals[:, :, 0:32], pk_t[:], 4, None,
                mybir.AluOpType.logical_shift_right)
            nc.vector.tensor_scalar(
                vals[:, :, 32:64], pk_t[:], 15, None,
                mybir.AluOpType.bitwise_and)
            inv_t = deq.tile([P, G], BF16, tag="inv")
            with nc.allow_low_precision(reason="4-bit dequant; bf16 ample"):
                nc.vector.reciprocal(inv_t[:], sc_all[:, nt])
            wbf = deq.tile([P, G, GS], BF16, tag="wbf")
            wt = wres.tile([P, KT, P], BF16, tag="wt")
            # affine + transpose in two G-halves so the first k-tiles of the
            # weight tile land ~5us earlier (the matmul chain reads k-tile
            # slices and can start on the first half)
            H = G // 2
            for h in range(2):
                gs_ = slice(h * H, (h + 1) * H)
                inv_b = inv_t[:, gs_].unsqueeze(2).broadcast_to([P, H, GS])
                nc.vector.tensor_tensor(wbf[:, gs_], vals[:, gs_], inv_b,
                                        mybir.AluOpType.mult)
                mn_b = mn_all[:, nt, gs_].unsqueeze(2).broadcast_to([P, H, GS])
                nc.vector.tensor_tensor(wbf[:, gs_], wbf[:, gs_], mn_b,
                                        mybir.AluOpType.add)
                nc.scalar.dma_start_transpose(
                    wt[:, h * KT // 2:(h + 1) * KT // 2, :],
                    wbf[:, gs_].rearrange("p g j -> p (g j)"))
            return wt

        def mm_block(wt, xq, q, nt):
            pt = psum.tile([P, MB], FP32, tag="pt")
            for kt in range(KT):
                nc.tensor.matmul(pt[:], lhsT=wt[:, kt, :],
                                 rhs=xq[:, kt, :],
                                 start=(kt == 0), stop=(kt == KT - 1))
            ot = outp.tile([P, MB], FP32, tag="ot")
            nc.scalar.activation(ot[:], pt[:],
                                 mybir.ActivationFunctionType.Identity,
                                 bias=bias_pt[:, nt:nt + 1])
            nc.scalar.dma_start(
                out=out_d[nt * P:(nt + 1) * P, q * MB:(q + 1) * MB],
                in_=ot[:])

        # Phase 1: n-tile-major over the first two token blocks (PE consumes
        # ~13.6us per fresh weight tile, matching the dequant cadence).
        wts = []
        for nt in range(NT):
            wts.append(dequant_ntile(nt))
            mm_block(wts[nt], xqs[0], 0, nt)
            mm_block(wts[nt], xqs[1], 1, nt)
            if nt == 3:
                xqs[2] = load_x(2)

        # Phase 2: token-block-major over the resident weight tiles.
        xq_cur = xqs[2]
        for q in range(2, QT):
            xq_next = load_x(q + 1) if q + 1 < QT else None
            for nt in range(NT):
                mm_block(wts[nt], xq_cur, q, nt)
            xq_cur = xq_next


_CACHED = {}


def _build():
    if "nc" in _CACHED:
        return _CACHED["nc"]
    nc = bacc.Bacc("TRN2", target_bir_lowering=False, debug=False)
    tensors = {
        "x": nc.dram_tensor("x", [IN, M_CORE], BF16, kind="ExternalInput"),
        "packed": nc.dram_tensor("packed", [N_CORE, IN // GS, GS // 2], U8,
                                 kind="ExternalInput"),
        "mn": nc.dram_tensor("mn", [N_CORE, IN // GS], FP32,
                             kind="ExternalInput"),
        "scale": nc.dram_tensor("scale", [N_CORE, IN // GS], FP32,
                                kind="ExternalInput"),
        "bias": nc.dram_tensor("bias", [1, N_CORE], FP32,
                               kind="ExternalInput"),
        "out": nc.dram_tensor("out", [N_CORE, M_CORE], FP32,
                              kind="ExternalOutput"),
    }
    ins = {k: tensors[k].ap() for k in ("x", "packed", "mn", "scale", "bias")}
    outs = {"out": tensors["out"].ap()}
    with tile.TileContext(nc) as tc:
        _emit_kernel(tc, outs, ins, M=M_CORE, K=IN, N=N_CORE, MB=MB)
    nc.compile()
    _CACHED["nc"] = nc
    return nc


def kernel(x, packed, mn, scale, bias, _trace=False, _trace_kwargs=None):
    import ml_dtypes

    nc = _build()

    xf = np.asarray(x, np.float32).reshape(B * S, IN)
    xT = [
        np.ascontiguousarray(
            xf[r * M_CORE:(r + 1) * M_CORE].T).astype(ml_dtypes.bfloat16)
        for r in range(R_SHARDS)
    ]
    pk_u8 = np.asarray(packed).astype(np.uint8)  # byte values 0..255, lossless
    in_maps = []
    for r in range(R_SHARDS):
        for c in range(C_SHARDS):
            ns = slice(c * N_CORE, (c + 1) * N_CORE)
            in_maps.append({
                "x": xT[r],
                "packed": np.ascontiguousarray(pk_u8[ns]),
                "mn": np.ascontiguousarray(mn[ns, :, 0]),
                "scale": np.ascontiguousarray(scale[ns, :, 0]),
                "bias": np.ascontiguousarray(
                    bias[ns].reshape(1, N_CORE).astype(np.float32)),
            })

    res = run_bass_kernel_spmd(
        nc, in_maps, core_ids=list(range(R_SHARDS * C_SHARDS)),
        trace=_trace, **(_trace_kwargs or {}))

    out = np.empty((B * S, OUT), np.float32)
    for r in range(R_SHARDS):
        for c in range(C_SHARDS):
            shard = res.results[r * C_SHARDS + c]["out"]  # [N_CORE, M_CORE]
            out[r * M_CORE:(r + 1) * M_CORE,
                c * N_CORE:(c + 1) * N_CORE] = shard.T
    kernel.last_exec_time_ns = res.exec_time_ns
    kernel.last_profile = res.profile_json
    return out.reshape(B, S, OUT)
